# revision 1
# baseline (speedup 1.0000x reference)
"""Trainium2 Bass kernel for EquivariantProductBasisBlock (MACE-style symmetric contraction).

Math (per irrep L, per node n, channel c):
  T1[m,a,b] = sum_{i,p} U3[m,a,b,i,p] w3[e_n,p,c] x[n,c,i] + sum_p U2[m,a,b,p] w2[e_n,p,c]
  T2[m,a]   = sum_b (T1[m,a,b]) x[n,c,b]
  T3[m]     = sum_a (T2[m,a] + U1[m,a] w1[e_n,c]) x[n,c,a]
  out[n,d,m]= sum_c T3[n,c,m] Wlin[c,d] / sqrt(C);  concat irreps; + sc

Device mapping: one PE matmul per node with a per-node stationary
zT[(p,i)+w2+w1, c] (z = x*w3 built on DVE, transposed on PE) against a shared
moving operand U3cat[127, 396] whose columns are (m,a)*10+b plus 36 U1 columns.
Stages 2/3 are DVE multiply + segmented reduce; Wlin is a K=128 matmul.
Data-parallel over nodes across 8 cores; element-gathered weights are prepared
host-side (equivalent to the reference's one-hot einsum gather).
"""

import sys

sys.path.insert(0, "/opt/trn_rl_repo")

import numpy as np

import concourse.bacc as bacc
import concourse.mybir as mybir
import concourse.tile as tile
from concourse.bass_utils import run_bass_kernel_spmd

# Problem constants (hardcoded per harness contract)
N, C, I, E = 4096, 128, 9, 10
NCORES = 8
NN = N // NCORES          # nodes per core = 512
IP = 10                   # i padded to even 10
NP3 = 12                  # total cubic paths (5 L0 + 7 L1)
KZ = NP3 * IP             # 120 z rows
KW2 = 5                   # w2 rows (2 L0 + 3 L1)
KW1 = 2                   # w1 rows
K = KZ + KW2 + KW1        # 127 stationary rows
NMA = 36                  # (m,a) pairs: 9 L0 + 27 L1
F1 = NMA * IP + NMA       # 396 matmul output cols: T1 (360) + U1w (36)
GRP = 16                  # nodes per DMA group
BLK = 128                 # nodes per Wlin/output block
FP32 = mybir.dt.float32
BF16 = mybir.dt.bfloat16
F32R = mybir.dt.float32r
INV_SQRT_C = 1.0 / np.sqrt(C)

_CACHE = {}


def _build_u3cat(U3_0, U2_0, U1_0, U3_1, U2_1, U1_1):
    """U3cat[k, f]: k = stationary row, f = (ma)*10+b for f<360, else 360+(ma)."""
    u = np.zeros((K, F1), np.float32)
    # z rows: k = p*10 + i ; cols (ma)*10 + b
    zblk = np.zeros((NP3, IP, NMA, IP), np.float32)   # [p, i, ma, b]
    # L0: U3_0 [1, a, b, i, p] -> [p, i, a, b]
    zblk[0:5, 0:9, 0:9, 0:9] = U3_0[0].transpose(3, 2, 0, 1)
    # L1: U3_1 [m, a, b, i, p] -> [p, i, (m,a), b]
    zblk[5:12, 0:9, 9:36, 0:9] = U3_1.transpose(4, 3, 0, 1, 2).reshape(7, 9, 27, 9)
    u[:KZ, : NMA * IP] = zblk.reshape(KZ, NMA * IP)
    # w2 rows
    w2blk = np.zeros((KW2, NMA, IP), np.float32)
    w2blk[0:2, 0:9, 0:9] = U2_0[0].transpose(2, 0, 1)          # [p2, a, b]
    w2blk[2:5, 9:36, 0:9] = U2_1.transpose(3, 0, 1, 2).reshape(3, 27, 9)
    u[KZ : KZ + KW2, : NMA * IP] = w2blk.reshape(KW2, NMA * IP)
    # w1 rows -> U1 columns
    u[KZ + KW2, NMA * IP : NMA * IP + 9] = U1_0[0, :, 0]
    u[KZ + KW2 + 1, NMA * IP + 9 :] = U1_1[:, :, 0].reshape(27)
    return u


def _build_program(repeat: int = 1):
    nc = bacc.Bacc(
        "TRN2",
        target_bir_lowering=False,
        debug=False,
        enable_asserts=False,
        num_devices=NCORES,
    )

    NGRP = NN // GRP      # 32 groups per core
    NBLK = NN // BLK      # 4 blocks per core
    GPB = BLK // GRP      # 8 groups per block

    x_d = nc.dram_tensor("x10", [NGRP, 128, GRP * IP], FP32, kind="ExternalInput").ap()
    w_d = nc.dram_tensor("wsel", [NGRP, 128, GRP * 19], FP32, kind="ExternalInput").ap()
    sc_d = nc.dram_tensor("scin", [NN, 512], FP32, kind="ExternalInput").ap()
    u3_d = nc.dram_tensor("u3cat", [K, F1], F32R, kind="ExternalInput").ap()
    wl0_d = nc.dram_tensor("wl0", [128, 128], FP32, kind="ExternalInput").ap()
    wl1_d = nc.dram_tensor("wl1", [128, 128], FP32, kind="ExternalInput").ap()
    id_d = nc.dram_tensor("ident", [128, 128], FP32, kind="ExternalInput").ap()
    out_d = nc.dram_tensor("out", [NN, 512], FP32, kind="ExternalOutput").ap()

    with tile.TileContext(nc) as tc:
        with (
            tc.tile_pool(name="const", bufs=1) as cpool,
            tc.tile_pool(name="io", bufs=3) as iopool,
            tc.tile_pool(name="work", bufs=3) as wpool,
            tc.tile_pool(name="small", bufs=4) as spool,
            tc.tile_pool(name="blkbuf", bufs=2) as bpool,
            tc.tile_pool(name="zt_ps", bufs=1, space="PSUM") as ztps,
            tc.tile_pool(name="p1_ps", bufs=1, space="PSUM") as p1ps,
            tc.tile_pool(name="lin_ps", bufs=1, space="PSUM") as linps,
            tc.tile_pool(name="ot_ps", bufs=1, space="PSUM") as otps,
        ):
            u3cat = cpool.tile([K, F1], F32R)
            nc.sync.dma_start(out=u3cat[:], in_=u3_d[:])
            wl0 = cpool.tile([128, 128], FP32)
            nc.sync.dma_start(out=wl0[:], in_=wl0_d[:])
            wl1 = cpool.tile([128, 128], FP32)
            nc.sync.dma_start(out=wl1[:], in_=wl1_d[:])
            ident = cpool.tile([128, 128], FP32)
            nc.sync.dma_start(out=ident[:], in_=id_d[:])

            for _rep in range(repeat):
                for blk in range(NBLK):
                    t3buf = bpool.tile([128, BLK * 4], FP32, tag="t3buf")
                    for gi in range(GPB):
                        g = blk * GPB + gi
                        xg = iopool.tile([128, GRP, IP], FP32, tag="xg")
                        nc.sync.dma_start(
                            out=xg[:], in_=x_d[g].rearrange("c (n i) -> c n i", i=IP)
                        )
                        xgb = iopool.tile([128, GRP, IP], BF16, tag="xgb")
                        nc.gpsimd.dma_start(
                            out=xgb[:], in_=x_d[g].rearrange("c (n i) -> c n i", i=IP)
                        )
                        wg = iopool.tile([128, GRP, 19], FP32, tag="wg")
                        nc.sync.dma_start(
                            out=wg[:], in_=w_d[g].rearrange("c (n w) -> c n w", w=19)
                        )
                        # batched z build for all 16 nodes
                        zf = wpool.tile([128, GRP, K], FP32, tag="zf")
                        nc.vector.tensor_mul(
                            out=zf[:, :, 0:KZ].rearrange(
                                "c n (p i) -> c n p i", i=IP
                            ),
                            in0=xg[:].unsqueeze(2).broadcast_to([128, GRP, NP3, IP]),
                            in1=wg[:, :, 0:NP3].unsqueeze(3).broadcast_to(
                                [128, GRP, NP3, IP]
                            ),
                        )
                        nc.any.tensor_copy(out=zf[:, :, KZ:K], in_=wg[:, :, NP3:19])
                        pe1 = wpool.tile([128, GRP, F1], BF16, tag="pe1")
                        for q in range(GRP // 4):  # 4-node subgroups
                            ztp = ztps.tile([K, 4, 128], FP32, tag="ztp")
                            for j4 in range(4):
                                nc.tensor.transpose(
                                    ztp[:, j4], zf[:, q * 4 + j4], ident[:]
                                )
                            ztsb = wpool.tile([K, 4, 128], F32R, tag="ztsb")
                            nc.any.tensor_copy(out=ztsb[:], in_=ztp[:])
                            # 4 matmuls into one 4-bank PSUM tile (bank-aligned
                            # 512-f32 slots), one evacuation copy for all 4
                            p1 = p1ps.tile([128, 4, 512], FP32, tag="p1")
                            for j4 in range(4):
                                nc.tensor.matmul(
                                    p1[:, j4, 0:F1], ztsb[:, j4], u3cat[:]
                                )
                            nc.any.tensor_copy(
                                out=pe1[:, q * 4 : q * 4 + 4],
                                in_=p1[:, :, 0:F1],
                            )
                        # batched stage 2 over all 16 nodes
                        m1 = wpool.tile([128, GRP, NMA, IP], BF16, tag="m1")
                        nc.vector.tensor_mul(
                            out=m1[:],
                            in0=pe1[:, :, 0 : NMA * IP].rearrange(
                                "c n (ma b) -> c n ma b", b=IP
                            ),
                            in1=xgb[:].unsqueeze(2).broadcast_to(
                                [128, GRP, NMA, IP]
                            ),
                        )
                        t2 = spool.tile([128, GRP, NMA], FP32, tag="t2")
                        nc.vector.tensor_reduce(
                            out=t2[:],
                            in_=m1[:],
                            axis=mybir.AxisListType.X,
                            op=mybir.AluOpType.add,
                        )
                        # batched stage 3
                        s = spool.tile([128, GRP, NMA], BF16, tag="s")
                        nc.vector.tensor_add(
                            out=s[:], in0=t2[:], in1=pe1[:, :, NMA * IP : F1]
                        )
                        s2 = spool.tile([128, GRP, NMA], BF16, tag="s2")
                        nc.vector.tensor_mul(
                            out=s2[:].rearrange("c n (m a) -> c n m a", a=9),
                            in0=s[:].rearrange("c n (m a) -> c n m a", a=9),
                            in1=xgb[:, :, 0:9].unsqueeze(2).broadcast_to(
                                [128, GRP, 4, 9]
                            ),
                        )
                        node0 = gi * GRP  # within block
                        nc.vector.tensor_reduce(
                            out=t3buf[
                                :, node0 * 4 : node0 * 4 + 4 * GRP
                            ].rearrange("c (n m) -> c n m", m=4),
                            in_=s2[:].rearrange("c n (m a) -> c n m a", a=9),
                            axis=mybir.AxisListType.X,
                            op=mybir.AluOpType.add,
                        )
                    # Wlin over channels for the 128-node block
                    t3v = t3buf[:].rearrange("c (n k) -> c n k", k=4)
                    lin0 = linps.tile([128, BLK], FP32, tag="lin0")
                    nc.tensor.matmul(lin0[:], wl0[:], t3v[:, :, 0])
                    lin1 = linps.tile([128, BLK, 3], FP32, tag="lin1")
                    nc.tensor.matmul(lin1[:], wl1[:], t3v[:, :, 1:4])
                    lin0sb = bpool.tile([128, BLK], FP32, tag="lin0sb")
                    nc.any.tensor_scalar_mul(lin0sb[:], lin0[:], INV_SQRT_C)
                    lin1sb = bpool.tile([128, BLK * 3], FP32, tag="lin1sb")
                    nc.any.tensor_scalar_mul(
                        lin1sb[:], lin1[:].rearrange("d n m -> d (n m)"), INV_SQRT_C
                    )
                    scg = bpool.tile([128, 512], FP32, tag="scg")
                    nc.sync.dma_start(
                        out=scg[:], in_=sc_d[blk * BLK : (blk + 1) * BLK]
                    )
                    outb = bpool.tile([128, 512], FP32, tag="outb")
                    ot0 = otps.tile([128, 128], FP32, tag="ot")
                    nc.tensor.transpose(ot0[:], lin0sb[:], ident[:])
                    nc.vector.tensor_add(
                        out=outb[:, 0:128], in0=ot0[:], in1=scg[:, 0:128]
                    )
                    for m in range(3):
                        otm = otps.tile([128, 128], FP32, tag="ot")
                        nc.tensor.transpose(
                            otm[:],
                            lin1sb[:].rearrange("d (n m) -> d n m", m=3)[:, :, m],
                            ident[:],
                        )
                        dst = outb[:, 128:512].rearrange(
                            "n (d m) -> n d m", m=3
                        )[:, :, m]
                        src = scg[:, 128:512].rearrange(
                            "n (d m) -> n d m", m=3
                        )[:, :, m]
                        nc.vector.tensor_add(out=dst, in0=otm[:], in1=src)
                    nc.sync.dma_start(
                        out=out_d[blk * BLK : (blk + 1) * BLK], in_=outb[:]
                    )

    nc.compile()
    return nc


def _get_program(repeat: int = 1):
    key = f"nc{repeat}"
    if key not in _CACHE:
        _CACHE[key] = _build_program(repeat)
    return _CACHE[key]


def kernel(**inputs) -> np.ndarray:
    node_feats = np.asarray(inputs["node_feats"], np.float32)
    sc = np.asarray(inputs["sc"], np.float32)
    node_attrs = np.asarray(inputs["node_attrs"], np.float32)
    elem = np.argmax(node_attrs, axis=1)

    # element-gathered weights [N, C, 19]: 12 w3 cols, 5 w2 cols, 2 w1 cols
    wall = np.concatenate(
        [
            np.asarray(inputs["W3_0"], np.float32),
            np.asarray(inputs["W3_1"], np.float32),
            np.asarray(inputs["W2_0"], np.float32),
            np.asarray(inputs["W2_1"], np.float32),
            np.asarray(inputs["W1_0"], np.float32),
            np.asarray(inputs["W1_1"], np.float32),
        ],
        axis=1,
    )  # [E, 19, C]
    wsel = wall[elem].transpose(0, 2, 1)  # [N, C, 19]

    x10 = np.zeros((N, C, IP), np.float32)
    x10[:, :, :I] = node_feats

    u3cat = _build_u3cat(
        np.asarray(inputs["U3_0"], np.float32),
        np.asarray(inputs["U2_0"], np.float32),
        np.asarray(inputs["U1_0"], np.float32),
        np.asarray(inputs["U3_1"], np.float32),
        np.asarray(inputs["U2_1"], np.float32),
        np.asarray(inputs["U1_1"], np.float32),
    )
    ident = np.eye(128, dtype=np.float32)
    wl0 = np.asarray(inputs["Wlin_0"], np.float32)
    wl1 = np.asarray(inputs["Wlin_1"], np.float32)

    NGRP = NN // GRP
    in_maps = []
    for k in range(NCORES):
        lo, hi = k * NN, (k + 1) * NN
        # [NN, C, IP] -> [NGRP, C, GRP*IP] (group-major, channel-partition)
        xs = (
            x10[lo:hi]
            .reshape(NGRP, GRP, C, IP)
            .transpose(0, 2, 1, 3)
            .reshape(NGRP, C, GRP * IP)
        )
        ws = (
            wsel[lo:hi]
            .reshape(NGRP, GRP, C, 19)
            .transpose(0, 2, 1, 3)
            .reshape(NGRP, C, GRP * 19)
        )
        in_maps.append(
            {
                "x10": np.ascontiguousarray(xs),
                "wsel": np.ascontiguousarray(ws),
                "scin": np.ascontiguousarray(sc[lo:hi]),
                "u3cat": u3cat,
                "wl0": wl0,
                "wl1": wl1,
                "ident": ident,
            }
        )

    nc = _get_program()
    res = run_bass_kernel_spmd(nc, in_maps, core_ids=list(range(NCORES)))
    out = np.concatenate([res.results[k]["out"] for k in range(NCORES)], axis=0)
    return out.astype(np.float32)



# revision 2
# speedup vs baseline: 10.6733x; 10.6733x over previous
"""Trainium2 Bass kernel for EquivariantProductBasisBlock (MACE-style symmetric contraction).

Math (per irrep L, per node n, channel c):
  T1[m,a,b] = sum_{i,p} U3[m,a,b,i,p] w3[e_n,p,c] x[n,c,i] + sum_p U2[m,a,b,p] w2[e_n,p,c]
  T2[m,a]   = sum_b (T1[m,a,b]) x[n,c,b]
  T3[m]     = sum_a (T2[m,a] + U1[m,a] w1[e_n,c]) x[n,c,a]
  out[n,d,m]= sum_c T3[n,c,m] Wlin[c,d] / sqrt(C);  concat irreps; + sc

Device mapping: one PE matmul per node with a per-node stationary
zT[(p,i)+w2+w1, c] (z = x*w3 built on DVE, transposed on PE) against a shared
moving operand U3cat[127, 396] whose columns are (m,a)*10+b plus 36 U1 columns.
Stages 2/3 are DVE multiply + segmented reduce; Wlin is a K=128 matmul.
Data-parallel over nodes across 8 cores; element-gathered weights are prepared
host-side (equivalent to the reference's one-hot einsum gather).

The 32 per-group (16-node) iterations run in a tc.For_i hardware loop so the
group body appears once in the program: the NEFF/BIR stays ~60 instructions
per pipeline pass instead of ~2200, which shrinks both program-load overhead
and dispatch work while executing the identical math.
"""

import sys

sys.path.insert(0, "/opt/trn_rl_repo")

import numpy as np

import concourse.bacc as bacc
import concourse.mybir as mybir
import concourse.tile as tile
from concourse.bass import ts
from concourse.bass_utils import run_bass_kernel_spmd

# Problem constants (hardcoded per harness contract)
N, C, I, E = 4096, 128, 9, 10
NCORES = 8
NN = N // NCORES          # nodes per core = 512
IP = 10                   # i padded to even 10
NP3 = 12                  # total cubic paths (5 L0 + 7 L1)
KZ = NP3 * IP             # 120 z rows
KW2 = 5                   # w2 rows (2 L0 + 3 L1)
KW1 = 2                   # w1 rows
K = KZ + KW2 + KW1        # 127 stationary rows
NMA = 36                  # (m,a) pairs: 9 L0 + 27 L1
F1 = NMA * IP + NMA       # 396 matmul output cols: T1 (360) + U1w (36)
GRP = 16                  # nodes per group (one hw-loop iteration)
NGRP = NN // GRP          # 32 groups per core
XWC = IP + 19             # packed per-node input cols: 10 x + 19 w
FP32 = mybir.dt.float32
BF16 = mybir.dt.bfloat16
F32R = mybir.dt.float32r
INV_SQRT_C = 1.0 / np.sqrt(C)

_CACHE = {}


def _build_u3cat(U3_0, U2_0, U1_0, U3_1, U2_1, U1_1):
    """U3cat[k, f]: k = stationary row, f = (ma)*10+b for f<360, else 360+(ma)."""
    u = np.zeros((K, F1), np.float32)
    # z rows: k = p*10 + i ; cols (ma)*10 + b
    zblk = np.zeros((NP3, IP, NMA, IP), np.float32)   # [p, i, ma, b]
    # L0: U3_0 [1, a, b, i, p] -> [p, i, a, b]
    zblk[0:5, 0:9, 0:9, 0:9] = U3_0[0].transpose(3, 2, 0, 1)
    # L1: U3_1 [m, a, b, i, p] -> [p, i, (m,a), b]
    zblk[5:12, 0:9, 9:36, 0:9] = U3_1.transpose(4, 3, 0, 1, 2).reshape(7, 9, 27, 9)
    u[:KZ, : NMA * IP] = zblk.reshape(KZ, NMA * IP)
    # w2 rows
    w2blk = np.zeros((KW2, NMA, IP), np.float32)
    w2blk[0:2, 0:9, 0:9] = U2_0[0].transpose(2, 0, 1)          # [p2, a, b]
    w2blk[2:5, 9:36, 0:9] = U2_1.transpose(3, 0, 1, 2).reshape(3, 27, 9)
    u[KZ : KZ + KW2, : NMA * IP] = w2blk.reshape(KW2, NMA * IP)
    # w1 rows -> U1 columns
    u[KZ + KW2, NMA * IP : NMA * IP + 9] = U1_0[0, :, 0]
    u[KZ + KW2 + 1, NMA * IP + 9 :] = U1_1[:, :, 0].reshape(27)
    return u


def prepare_in_maps(inputs):
    """Host-side prep: element gather, i-padding, per-core packing."""
    node_feats = np.asarray(inputs["node_feats"], np.float32)
    sc = np.asarray(inputs["sc"], np.float32)
    node_attrs = np.asarray(inputs["node_attrs"], np.float32)
    elem = np.argmax(node_attrs, axis=1)

    # element-gathered weights [N, C, 19]: 12 w3 cols, 5 w2 cols, 2 w1 cols
    wall = np.concatenate(
        [
            np.asarray(inputs["W3_0"], np.float32),
            np.asarray(inputs["W3_1"], np.float32),
            np.asarray(inputs["W2_0"], np.float32),
            np.asarray(inputs["W2_1"], np.float32),
            np.asarray(inputs["W1_0"], np.float32),
            np.asarray(inputs["W1_1"], np.float32),
        ],
        axis=1,
    )  # [E, 19, C]
    wsel = wall[elem].transpose(0, 2, 1)  # [N, C, 19]

    xw = np.zeros((N, C, XWC), np.float32)
    xw[:, :, :I] = node_feats
    xw[:, :, IP:] = wsel

    u3cat = _build_u3cat(
        np.asarray(inputs["U3_0"], np.float32),
        np.asarray(inputs["U2_0"], np.float32),
        np.asarray(inputs["U1_0"], np.float32),
        np.asarray(inputs["U3_1"], np.float32),
        np.asarray(inputs["U2_1"], np.float32),
        np.asarray(inputs["U1_1"], np.float32),
    )
    ident = np.eye(128, dtype=np.float32)
    # fold the 1/sqrt(C) output scale into the linear weights
    wl0 = np.asarray(inputs["Wlin_0"], np.float32) * INV_SQRT_C
    wl1 = np.asarray(inputs["Wlin_1"], np.float32) * INV_SQRT_C

    in_maps = []
    for k in range(NCORES):
        lo, hi = k * NN, (k + 1) * NN
        # [NN, C, XWC] -> [NGRP*C, GRP*XWC] (group-major rows of 128 channels)
        xs = (
            xw[lo:hi]
            .reshape(NGRP, GRP, C, XWC)
            .transpose(0, 2, 1, 3)
            .reshape(NGRP * C, GRP * XWC)
        )
        in_maps.append(
            {
                "xw": np.ascontiguousarray(xs),
                "scin": np.ascontiguousarray(sc[lo:hi]),
                "u3cat": u3cat,
                "wl0": wl0,
                "wl1": wl1,
                "ident": ident,
            }
        )
    return in_maps


def _build_program(repeat: int = 1):
    nc = bacc.Bacc(
        "TRN2",
        target_bir_lowering=False,
        debug=False,
        enable_asserts=False,
        num_devices=NCORES,
    )

    xw_d = nc.dram_tensor("xw", [NGRP * 128, GRP * XWC], FP32, kind="ExternalInput").ap()
    sc_d = nc.dram_tensor("scin", [NN, 512], FP32, kind="ExternalInput").ap()
    u3_d = nc.dram_tensor("u3cat", [K, F1], F32R, kind="ExternalInput").ap()
    wl0_d = nc.dram_tensor("wl0", [128, 128], FP32, kind="ExternalInput").ap()
    wl1_d = nc.dram_tensor("wl1", [128, 128], FP32, kind="ExternalInput").ap()
    id_d = nc.dram_tensor("ident", [128, 128], FP32, kind="ExternalInput").ap()
    out_d = nc.dram_tensor("out", [NN, 512], FP32, kind="ExternalOutput").ap()

    with tile.TileContext(nc) as tc:
        with (
            tc.tile_pool(name="const", bufs=1) as cpool,
            tc.tile_pool(name="io", bufs=1) as iopool,
            tc.tile_pool(name="work", bufs=1) as wpool,
            tc.tile_pool(name="small", bufs=1) as spool,
            tc.tile_pool(name="zt_ps", bufs=1, space="PSUM") as ztps,
            tc.tile_pool(name="p1_ps", bufs=1, space="PSUM") as p1ps,
            tc.tile_pool(name="lin_ps", bufs=1, space="PSUM") as linps,
            tc.tile_pool(name="ot_ps", bufs=1, space="PSUM") as otps,
        ):
            u3cat = cpool.tile([K, F1], F32R)
            nc.sync.dma_start(out=u3cat[:], in_=u3_d[:])
            wl0 = cpool.tile([128, 128], FP32)
            nc.sync.dma_start(out=wl0[:], in_=wl0_d[:])
            wl1 = cpool.tile([128, 128], FP32)
            nc.sync.dma_start(out=wl1[:], in_=wl1_d[:])
            ident = cpool.tile([128, 128], FP32)
            nc.sync.dma_start(out=ident[:], in_=id_d[:])

            for _rep in range(repeat):
                with tc.For_i(0, NGRP) as g:
                    xwg = iopool.tile([128, GRP, XWC], FP32, tag="xwg")
                    nc.sync.dma_start(
                        out=xwg[:],
                        in_=xw_d[ts(g, 128)].rearrange("c (n w) -> c n w", w=XWC),
                    )
                    xg = xwg[:, :, 0:IP]
                    wg = xwg[:, :, IP:XWC]
                    xgb = iopool.tile([128, GRP, IP], BF16, tag="xgb")
                    nc.any.tensor_copy(out=xgb[:], in_=xg)
                    # batched z build for all 16 nodes
                    zf = wpool.tile([128, GRP, K], FP32, tag="zf")
                    nc.vector.tensor_mul(
                        out=zf[:, :, 0:KZ].rearrange("c n (p i) -> c n p i", i=IP),
                        in0=xg.unsqueeze(2).broadcast_to([128, GRP, NP3, IP]),
                        in1=wg[:, :, 0:NP3].unsqueeze(3).broadcast_to(
                            [128, GRP, NP3, IP]
                        ),
                    )
                    nc.any.tensor_copy(out=zf[:, :, KZ:K], in_=wg[:, :, NP3:])
                    pe1 = wpool.tile([128, GRP, F1], BF16, tag="pe1")
                    for q in range(GRP // 4):  # 4-node subgroups
                        ztp = ztps.tile([K, 4, 128], FP32, tag="ztp")
                        for j4 in range(4):
                            nc.tensor.transpose(
                                ztp[:, j4], zf[:, q * 4 + j4], ident[:]
                            )
                        ztsb = wpool.tile([K, 4, 128], F32R, tag="ztsb")
                        nc.any.tensor_copy(out=ztsb[:], in_=ztp[:])
                        # 4 matmuls into one 4-bank PSUM tile (bank-aligned
                        # 512-f32 slots), one evacuation copy for all 4
                        p1 = p1ps.tile([128, 4, 512], FP32, tag="p1")
                        for j4 in range(4):
                            nc.tensor.matmul(p1[:, j4, 0:F1], ztsb[:, j4], u3cat[:])
                        nc.any.tensor_copy(
                            out=pe1[:, q * 4 : q * 4 + 4], in_=p1[:, :, 0:F1]
                        )
                    # batched stage 2 over all 16 nodes
                    m1 = wpool.tile([128, GRP, NMA, IP], BF16, tag="m1")
                    nc.vector.tensor_mul(
                        out=m1[:],
                        in0=pe1[:, :, 0 : NMA * IP].rearrange(
                            "c n (ma b) -> c n ma b", b=IP
                        ),
                        in1=xgb[:].unsqueeze(2).broadcast_to([128, GRP, NMA, IP]),
                    )
                    t2 = spool.tile([128, GRP, NMA], FP32, tag="t2")
                    nc.vector.tensor_reduce(
                        out=t2[:],
                        in_=m1[:],
                        axis=mybir.AxisListType.X,
                        op=mybir.AluOpType.add,
                    )
                    # batched stage 3
                    s = spool.tile([128, GRP, NMA], BF16, tag="s")
                    nc.vector.tensor_add(
                        out=s[:], in0=t2[:], in1=pe1[:, :, NMA * IP : F1]
                    )
                    s2 = spool.tile([128, GRP, NMA], BF16, tag="s2")
                    nc.vector.tensor_mul(
                        out=s2[:].rearrange("c n (m a) -> c n m a", a=9),
                        in0=s[:].rearrange("c n (m a) -> c n m a", a=9),
                        in1=xgb[:, :, 0:9].unsqueeze(2).broadcast_to(
                            [128, GRP, 4, 9]
                        ),
                    )
                    t3 = spool.tile([128, GRP, 4], FP32, tag="t3")
                    nc.vector.tensor_reduce(
                        out=t3[:],
                        in_=s2[:].rearrange("c n (m a) -> c n m a", a=9),
                        axis=mybir.AxisListType.X,
                        op=mybir.AluOpType.add,
                    )
                    # Wlin over channels (scale prefolded into wl0/wl1)
                    lin = linps.tile([128, 4, GRP], FP32, tag="lin")
                    nc.tensor.matmul(lin[:, 0], wl0[:], t3[:, :, 0])
                    nc.tensor.matmul(
                        lin[:, 1:4],
                        wl1[:],
                        t3[:, :, 1:4].rearrange("c n m -> c m n"),
                    )
                    linsb = spool.tile([128, 4, GRP], FP32, tag="linsb")
                    nc.any.tensor_copy(out=linsb[:], in_=lin[:])
                    scg = spool.tile([GRP, 512], FP32, tag="scg")
                    nc.sync.dma_start(out=scg[:], in_=sc_d[ts(g, GRP)])
                    ot = otps.tile([GRP, 4, 128], FP32, tag="ot")
                    for j in range(4):
                        nc.tensor.transpose(ot[:, j], linsb[:, j], ident[:])
                    outg = spool.tile([GRP, 512], FP32, tag="outg")
                    nc.vector.tensor_add(
                        out=outg[:, 0:128], in0=ot[:, 0], in1=scg[:, 0:128]
                    )
                    nc.vector.tensor_add(
                        out=outg[:, 128:512].rearrange("n (d m) -> n d m", m=3),
                        in0=ot[:, 1:4].rearrange("n m d -> n d m"),
                        in1=scg[:, 128:512].rearrange("n (d m) -> n d m", m=3),
                    )
                    nc.sync.dma_start(out=out_d[ts(g, GRP)], in_=outg[:])

    nc.compile()
    return nc


def _get_program(repeat: int = 1):
    key = f"nc{repeat}"
    if key not in _CACHE:
        _CACHE[key] = _build_program(repeat)
    return _CACHE[key]


def kernel(**inputs) -> np.ndarray:
    in_maps = prepare_in_maps(inputs)
    nc = _get_program()
    res = run_bass_kernel_spmd(nc, in_maps, core_ids=list(range(NCORES)))
    out = np.concatenate([res.results[k]["out"] for k in range(NCORES)], axis=0)
    return out.astype(np.float32)
